# revision 7
# baseline (speedup 1.0000x reference)
"""Trainium2 Bass kernel for GPT-style attention block (B=2, S=2048, D=1024, H=16).

Sharding: tensor-parallel over heads, 2 heads per core (8 cores).
Each core computes qkv for its heads, causal softmax attention, its partial
output projection (contracting only its 128 head-dims); host sums the 8
partial projections and concatenates head-sharded attn/present outputs.

Precision: float32r (fast fp32, 2 cyc/row) for qkv/scores/attn outputs;
bf16 (1 cyc/row) for the attn@v and projection matmuls.
Causal structure: only lower-triangular 512-col chunks are computed; the
strictly-upper region relies on pre-zeroed output buffers.
"""
import os
import numpy as np

import concourse.bacc as bacc
import concourse.mybir as mybir
import concourse.tile as tile
from concourse.bass_utils import run_bass_kernel_spmd
from concourse.masks import make_identity

B, S, D, H = 2, 2048, 1024, 16
HD = D // H            # 64
N_CORES = 8
HPC = H // N_CORES     # 2 heads per core
PD = HPC * HD          # 128 partition dims per core
F32 = mybir.dt.float32
F32R = mybir.dt.float32r
BF16 = mybir.dt.bfloat16
AF = mybir.ActivationFunctionType

_CACHE = {}
_last_in_maps = None


def _build():
    nc = bacc.Bacc(None, target_bir_lowering=False)

    x_ext = nc.declare_dram_parameter("x", [B, S, D], F32, isOutput=False)
    wqkv_ext = nc.declare_dram_parameter("wqkv", [D, 3 * PD], F32, isOutput=False)
    bqkv_ext = nc.declare_dram_parameter("bqkv", [3, PD], F32, isOutput=False)
    wp_ext = nc.declare_dram_parameter("wp", [PD, D], F32, isOutput=False)
    bp_ext = nc.declare_dram_parameter("bp", [1, D], F32, isOutput=False)
    attn_ext = nc.declare_dram_parameter("attn", [B, HPC, S, S], F32, isOutput=True)
    pres_ext = nc.declare_dram_parameter("present", [2, B, HPC, S, HD], F32, isOutput=True)
    apart_ext = nc.declare_dram_parameter("a_part", [B, S, D], F32, isOutput=True)

    with tile.TileContext(nc) as tc:
        with (
            tc.tile_pool(name="cst", bufs=1) as cst,
            tc.tile_pool(name="sb", bufs=1) as sb,
            tc.tile_pool(name="ps", bufs=1, space="PSUM") as ps,
        ):
            # ---- constants ----
            ident = cst.tile([128, 128], F32)
            make_identity(nc, ident[:])
            ident_r = cst.tile([128, 128], F32R)
            nc.vector.tensor_copy(ident_r[:], ident[:])

            # causal masks for the diagonal 512-chunk, one per row-block
            # position p in its group: keep where col' <= 128p + row'
            masks = []
            for p in range(4):
                m = cst.tile([128, 512], F32, name=f"mask{p}")
                nc.gpsimd.memset(m[:], 0.0)
                nc.gpsimd.affine_select(
                    out=m[:], in_=m[:],
                    compare_op=mybir.AluOpType.is_ge,
                    fill=-1e9, base=128 * p,
                    pattern=[[-1, 512]], channel_multiplier=1,
                )
                masks.append(m)

            ones_f = cst.tile([1, 128], F32)
            nc.gpsimd.memset(ones_f[:], 1.0)
            ones_b = cst.tile([1, 128], BF16)
            nc.vector.tensor_copy(ones_b[:], ones_f[:])

            # qkv weights: 8 k-blocks x 3 col-tiles, cast to f32r
            wsb = [[None] * 3 for _ in range(8)]
            for k in range(8):
                for t in range(3):
                    wf = sb.tile([128, 128], F32, tag="wldf", bufs=4, name=f"wf{k}_{t}")
                    nc.sync.dma_start(wf[:], wqkv_ext[128 * k:128 * (k + 1), 128 * t:128 * (t + 1)])
                    wr = cst.tile([128, 128], F32R, name=f"wr{k}_{t}")
                    nc.vector.tensor_copy(wr[:], wf[:])
                    wsb[k][t] = wr

            bq_sb = cst.tile([128, 3], F32)
            for t in range(3):
                nc.sync.dma_start(bq_sb[:, t:t + 1], bqkv_ext[t][:, None])

            wp_f = sb.tile([PD, D], F32, tag="wpf", bufs=1)
            nc.sync.dma_start(wp_f[:], wp_ext[:])
            wpb = cst.tile([PD, D], BF16)
            nc.vector.tensor_copy(wpb[:], wp_f[:])

            bp_f = sb.tile([1, D], F32, tag="bpf", bufs=1)
            nc.sync.dma_start(bp_f[:], bp_ext[:])
            bp_b = cst.tile([1, D], BF16)
            nc.vector.tensor_copy(bp_b[:], bp_f[:])

            cpi = [0]

            def cp(out, in_):
                # alternate psum->sbuf copies between ACT and DVE
                if cpi[0] % 2 == 0:
                    nc.scalar.copy(out, in_)
                else:
                    nc.vector.tensor_copy(out, in_)
                cpi[0] += 1

            for b in range(B):
                # ---- qkv projection (transposed layout) ----
                qT = sb.tile([128, S], F32R, tag="qT", bufs=2, name=f"qT{b}")
                kT = sb.tile([128, S], F32R, tag="kT", bufs=2, name=f"kT{b}")
                vrows = []
                for n in range(4):
                    xrows = []
                    for r in range(4):
                        xr = sb.tile([128, D], F32, tag="xrow", bufs=6, name=f"xr{b}_{n}_{r}")
                        nc.sync.dma_start(xr[:], x_ext[b, 512 * n + 128 * r:512 * n + 128 * (r + 1), :])
                        xrows.append(xr)
                    xts = []
                    for kp in range(4):          # pairs of d-blocks
                        pst = ps.tile([128, 1024], F32, tag="mm", bufs=2, name=f"pxt{b}_{n}_{kp}")
                        for half in range(2):
                            k = 2 * kp + half
                            for r in range(4):
                                nc.tensor.transpose(
                                    pst[:, 512 * half + 128 * r:512 * half + 128 * (r + 1)],
                                    xrows[r][:, 128 * k:128 * (k + 1)], ident[:])
                        xt = sb.tile([128, 1024], F32R, tag="xt", bufs=6, name=f"xt{b}_{n}_{kp}")
                        cp(xt[:], pst[:])
                        xts.append(xt)

                    def xtk(k):
                        return xts[k // 2][:, 512 * (k % 2):512 * (k % 2 + 1)]

                    # q,k into qT/kT (paired psum); v into vt chunk
                    psq = ps.tile([128, 1024], F32, tag="mm", bufs=2, name=f"pqa{b}_{n}")
                    for t in range(2):
                        for k in range(8):
                            nc.tensor.matmul(psq[:, 512 * t:512 * (t + 1)],
                                             wsb[k][t][:], xtk(k),
                                             start=(k == 0), stop=(k == 7))
                    nc.scalar.activation(out=qT[:, 512 * n:512 * (n + 1)], in_=psq[:, 0:512],
                                         func=AF.Identity, bias=bq_sb[:, 0:1], scale=1.0)
                    nc.scalar.activation(out=kT[:, 512 * n:512 * (n + 1)], in_=psq[:, 512:1024],
                                         func=AF.Identity, bias=bq_sb[:, 1:2], scale=1.0)
                    psqv = ps.tile([128, 512], F32, tag="tp", bufs=2, name=f"pqv{b}_{n}")
                    for k in range(8):
                        nc.tensor.matmul(psqv[:], wsb[k][2][:], xtk(k),
                                         start=(k == 0), stop=(k == 7))
                    vt = sb.tile([128, 512], F32R, tag="vt", bufs=3, name=f"vt{b}_{n}")
                    nc.scalar.activation(out=vt[:], in_=psqv[:], func=AF.Identity,
                                         bias=bq_sb[:, 2:3], scale=1.0)

                    # k/v row-major blocks (present outputs; v also feeds av as bf16)
                    for r in range(4):
                        sbi = 4 * n + r
                        psk = ps.tile([128, 128], F32R, tag="tp", bufs=2, name=f"psk{b}_{sbi}")
                        nc.tensor.transpose(psk[:], kT[:, 128 * sbi:128 * (sbi + 1)], ident_r[:])
                        krow = sb.tile([128, 128], F32R, tag="krow", bufs=4, name=f"kr{b}_{sbi}")
                        cp(krow[:], psk[:])
                        nc.sync.dma_start(
                            pres_ext[0, b, :, 128 * sbi:128 * (sbi + 1), :].rearrange("h s d -> s h d"),
                            krow[:].bitcast(F32).rearrange("p (h d) -> p h d", h=2))
                        psv2 = ps.tile([128, 128], F32R, tag="tp", bufs=2, name=f"psv{b}_{sbi}")
                        nc.tensor.transpose(psv2[:], vt[:, 128 * r:128 * (r + 1)], ident_r[:])
                        vrow = sb.tile([128, 128], F32R, tag="krow", bufs=4, name=f"vr{b}_{sbi}")
                        cp(vrow[:], psv2[:])
                        nc.sync.dma_start(
                            pres_ext[1, b, :, 128 * sbi:128 * (sbi + 1), :].rearrange("h s d -> s h d"),
                            vrow[:].bitcast(F32).rearrange("p (h d) -> p h d", h=2))
                        vrow_b = sb.tile([128, 128], BF16, tag="vrowb", bufs=32, name=f"vb{b}_{sbi}")
                        nc.vector.tensor_copy(vrow_b[:], psv2[:])
                        vrows.append(vrow_b)

                # ---- attention ----
                avT = sb.tile([128, S], BF16, tag="avT", bufs=2, name=f"avT{b}")
                for g in range(4):
                    npair = (g + 2) // 2           # number of 1024-wide chunk pairs
                    for hh in range(2):
                        hs = 64 * hh
                        Es = {}
                        for r in range(4):
                            i = 4 * g + r
                            accs = []
                            for p in range(npair):
                                wid = min(1024, 512 * (g + 1) - 1024 * p)
                                pss = ps.tile([128, 1024], F32, tag="mm", bufs=2,
                                              name=f"pss{b}_{hh}_{i}_{p}")
                                for half in range(wid // 512):
                                    ch = 2 * p + half
                                    nc.tensor.matmul(
                                        pss[:, 512 * half:512 * (half + 1)],
                                        qT[hs:hs + 64, 128 * i:128 * (i + 1)],
                                        kT[hs:hs + 64, 512 * ch:512 * (ch + 1)],
                                        start=True, stop=True)
                                    if ch == g:
                                        nc.vector.tensor_add(
                                            pss[:, 512 * half:512 * (half + 1)],
                                            pss[:, 512 * half:512 * (half + 1)], masks[r][:])
                                E = sb.tile([128, 1024], F32R, tag="E", bufs=10,
                                            name=f"E{b}_{hh}_{i}_{p}")
                                acc = sb.tile([128, 1], F32, tag="acc", bufs=10,
                                              name=f"ac{b}_{hh}_{i}_{p}")
                                nc.scalar.activation(out=E[:, :wid], in_=pss[:, :wid],
                                                     func=AF.Exp, scale=0.125,
                                                     accum_out=acc[:])
                                Es[(r, p)] = E
                                accs.append(acc)
                            if len(accs) == 1:
                                rsum = accs[0]
                            else:
                                rsum = sb.tile([128, 1], F32, tag="rs", bufs=4,
                                               name=f"rs{b}_{hh}_{i}")
                                nc.vector.tensor_add(rsum[:], accs[0][:], accs[1][:])
                            rcp = sb.tile([128, 1], F32, tag="rcp", bufs=6,
                                          name=f"rc{b}_{hh}_{i}")
                            nc.vector.reciprocal(rcp[:], rsum[:])
                            for p in range(npair):
                                wid = min(1024, 512 * (g + 1) - 1024 * p)
                                E = Es[(r, p)]
                                nc.vector.tensor_scalar_mul(E[:, :wid], E[:, :wid], rcp[:])
                                nc.sync.dma_start(
                                    attn_ext[b, hh, 128 * i:128 * (i + 1),
                                             1024 * p:1024 * p + wid],
                                    E[:, :wid].bitcast(F32))
                        # attn @ v (transposed layout, bf16 matmul)
                        avp = ps.tile([64, 512], F32, tag="av", bufs=2, name=f"avp{b}_{hh}_{g}")
                        njs = 4 * (g + 1)
                        for j in range(njs):
                            p, sub = j // 8, j % 8
                            pst = ps.tile([128, 512], F32R, tag="tp", bufs=2,
                                          name=f"pt{b}_{hh}_{g}_{j}")
                            for r in range(4):
                                nc.tensor.transpose(
                                    pst[:, 128 * r:128 * (r + 1)],
                                    Es[(r, p)][:, 128 * sub:128 * (sub + 1)], ident_r[:])
                            R = sb.tile([128, 512], BF16, tag="R", bufs=3,
                                        name=f"R{b}_{hh}_{g}_{j}")
                            cp(R[:], pst[:])
                            nc.tensor.matmul(avp[:], vrows[j][:, hs:hs + 64], R[:],
                                             start=(j == 0), stop=(j == njs - 1))
                        nc.scalar.copy(avT[hs:hs + 64, 512 * g:512 * (g + 1)], avp[:])
                    # ---- partial projection for this row group (bf16) ----
                    for mi in range(4):
                        m = 4 * g + mi
                        psp = ps.tile([128, 1024], F32, tag="mm", bufs=2,
                                      name=f"pp{b}_{g}_{mi}")
                        for nn2 in range(2):
                            nc.tensor.matmul(psp[:, 512 * nn2:512 * (nn2 + 1)],
                                             avT[:, 128 * m:128 * (m + 1)],
                                             wpb[:, 512 * nn2:512 * (nn2 + 1)],
                                             start=True, stop=False)
                            nc.tensor.matmul(psp[:, 512 * nn2:512 * (nn2 + 1)],
                                             ones_b[:],
                                             bp_b[:, 512 * nn2:512 * (nn2 + 1)],
                                             start=False, stop=True)
                        ao = sb.tile([128, 1024], F32, tag="ao", bufs=4,
                                     name=f"ao{b}_{g}_{mi}")
                        cp(ao[:], psp[:])
                        nc.sync.dma_start(apart_ext[b, 128 * m:128 * (m + 1), :], ao[:])
    nc.compile()
    return nc


def _get_nc():
    if "nc" not in _CACHE:
        _CACHE["nc"] = _build()
    return _CACHE["nc"]


def kernel(x, w_attn, b_attn, w_proj, b_proj):
    global _last_in_maps
    x = np.ascontiguousarray(np.asarray(x, dtype=np.float32))
    w_attn = np.asarray(w_attn, dtype=np.float32)
    b_attn = np.asarray(b_attn, dtype=np.float32)
    w_proj = np.asarray(w_proj, dtype=np.float32)
    b_proj = np.asarray(b_proj, dtype=np.float32)

    nc = _get_nc()
    in_maps = []
    for c in range(N_CORES):
        lo, hi = PD * c, PD * (c + 1)
        wqkv = np.ascontiguousarray(np.concatenate(
            [w_attn[:, lo:hi], w_attn[:, D + lo:D + hi], w_attn[:, 2 * D + lo:2 * D + hi]],
            axis=1))
        bqkv = np.ascontiguousarray(np.stack(
            [b_attn[lo:hi], b_attn[D + lo:D + hi], b_attn[2 * D + lo:2 * D + hi]]))
        in_maps.append({
            "x": x,
            "wqkv": wqkv,
            "bqkv": bqkv,
            "wp": np.ascontiguousarray(w_proj[lo:hi, :]),
            "bp": (b_proj / N_CORES)[None, :].copy(),
        })

    _last_in_maps = in_maps
    res = run_bass_kernel_spmd(nc, in_maps, list(range(N_CORES)))
    rs = res.results

    attn = np.concatenate([r["attn"] for r in rs], axis=1)          # [B, H, S, S]
    present = np.concatenate([r["present"] for r in rs], axis=2)    # [2, B, H, S, HD]
    a = rs[0]["a_part"]
    for r in rs[1:]:
        a = a + r["a_part"]
    return a, present, attn


# revision 8
# speedup vs baseline: 1.0467x; 1.0467x over previous
"""Trainium2 Bass kernel for GPT-style attention block (B=2, S=2048, D=1024, H=16).

Sharding: tensor-parallel over heads, 2 heads per core (8 cores).
Each core computes qkv for its heads, causal softmax attention, its partial
output projection (contracting only its 128 head-dims); host sums the 8
partial projections and concatenates head-sharded attn/present outputs.

Precision: fp16 matmul operands (1 cyc/row on the PE, ~5e-4 rel err, fp32
PSUM accumulation); the softmax/attn-output path runs in fp32 (float32r).
Causal structure: only lower-triangular 512-col chunks are computed; the
strictly-upper region relies on pre-zeroed output buffers.
"""
import os
import numpy as np

import concourse.bacc as bacc
import concourse.mybir as mybir
import concourse.tile as tile
from concourse.bass_utils import run_bass_kernel_spmd
from concourse.masks import make_identity

B, S, D, H = 2, 2048, 1024, 16
HD = D // H            # 64
N_CORES = 8
HPC = H // N_CORES     # 2 heads per core
PD = HPC * HD          # 128 partition dims per core
F32 = mybir.dt.float32
F32R = mybir.dt.float32r
F16 = mybir.dt.float16
AF = mybir.ActivationFunctionType

_CACHE = {}
_last_in_maps = None


def _build():
    nc = bacc.Bacc(None, target_bir_lowering=False)

    x_ext = nc.declare_dram_parameter("x16", [B, S, D], F16, isOutput=False)
    wqkv_ext = nc.declare_dram_parameter("wqkv16", [D, 3 * PD], F16, isOutput=False)
    bqkv_ext = nc.declare_dram_parameter("bqkv", [3, PD], F32, isOutput=False)
    wp_ext = nc.declare_dram_parameter("wp16", [PD, D], F16, isOutput=False)
    bp_ext = nc.declare_dram_parameter("bp16", [1, D], F16, isOutput=False)
    attn_ext = nc.declare_dram_parameter("attn", [B, HPC, S, S], F32, isOutput=True)
    pres_ext = nc.declare_dram_parameter("present", [2, B, HPC, S, HD], F32, isOutput=True)
    apart_ext = nc.declare_dram_parameter("a_part", [B, S, D], F32, isOutput=True)

    with tile.TileContext(nc) as tc:
        with (
            tc.tile_pool(name="cst", bufs=1) as cst,
            tc.tile_pool(name="sb", bufs=1) as sb,
            tc.tile_pool(name="ps", bufs=1, space="PSUM") as ps,
        ):
            # ---- constants ----
            ident = cst.tile([128, 128], F32)
            make_identity(nc, ident[:])
            ident16 = cst.tile([128, 128], F16)
            nc.vector.tensor_copy(ident16[:], ident[:])
            ident_r = cst.tile([128, 128], F32R)
            nc.vector.tensor_copy(ident_r[:], ident[:])

            # causal masks for the diagonal 512-chunk, one per row-block
            # position p in its group: keep where col' <= 128p + row'
            masks = []
            for p in range(4):
                m = cst.tile([128, 512], F32, name=f"mask{p}")
                nc.gpsimd.memset(m[:], 0.0)
                nc.gpsimd.affine_select(
                    out=m[:], in_=m[:],
                    compare_op=mybir.AluOpType.is_ge,
                    fill=-1e9, base=128 * p,
                    pattern=[[-1, 512]], channel_multiplier=1,
                )
                masks.append(m)

            ones16 = cst.tile([1, 128], F16)
            nc.gpsimd.memset(ones16[:], 1.0)

            # qkv weights: 8 k-blocks x 3 col-tiles (fp16, direct DMA)
            wsb = [[None] * 3 for _ in range(8)]
            for k in range(8):
                for t in range(3):
                    wr = cst.tile([128, 128], F16, name=f"w{k}_{t}")
                    nc.sync.dma_start(wr[:], wqkv_ext[128 * k:128 * (k + 1), 128 * t:128 * (t + 1)])
                    wsb[k][t] = wr

            bq_sb = cst.tile([128, 3], F32)
            for t in range(3):
                nc.sync.dma_start(bq_sb[:, t:t + 1], bqkv_ext[t][:, None])

            wp16 = cst.tile([PD, D], F16)
            nc.sync.dma_start(wp16[:], wp_ext[:])
            bp16 = cst.tile([1, D], F16)
            nc.sync.dma_start(bp16[:], bp_ext[:])

            for b in range(B):
                # ---- qkv projection (transposed layout) ----
                qT = sb.tile([128, S], F16, tag="qT", bufs=2, name=f"qT{b}")
                kT = sb.tile([128, S], F16, tag="kT", bufs=2, name=f"kT{b}")
                vrows = []
                for n in range(4):
                    xrows = []
                    for r in range(4):
                        xr = sb.tile([128, D], F16, tag="xrow", bufs=6, name=f"xr{b}_{n}_{r}")
                        nc.sync.dma_start(xr[:], x_ext[b, 512 * n + 128 * r:512 * n + 128 * (r + 1), :])
                        xrows.append(xr)
                    xts = []
                    for kp in range(4):          # pairs of d-blocks
                        pst = ps.tile([128, 1024], F16, tag="mm", bufs=2, name=f"pxt{b}_{n}_{kp}")
                        for half in range(2):
                            k = 2 * kp + half
                            for r in range(4):
                                nc.tensor.transpose(
                                    pst[:, 512 * half + 128 * r:512 * half + 128 * (r + 1)],
                                    xrows[r][:, 128 * k:128 * (k + 1)], ident16[:])
                        xt = sb.tile([128, 1024], F16, tag="xt", bufs=6, name=f"xt{b}_{n}_{kp}")
                        nc.vector.tensor_copy(xt[:], pst[:])
                        xts.append(xt)

                    def xtk(k):
                        return xts[k // 2][:, 512 * (k % 2):512 * (k % 2 + 1)]

                    # q,k into qT/kT (paired psum); v into vt chunk
                    psq = ps.tile([128, 1024], F32, tag="mm", bufs=2, name=f"pqa{b}_{n}")
                    for t in range(2):
                        for k in range(8):
                            nc.tensor.matmul(psq[:, 512 * t:512 * (t + 1)],
                                             wsb[k][t][:], xtk(k),
                                             start=(k == 0), stop=(k == 7))
                    nc.scalar.activation(out=qT[:, 512 * n:512 * (n + 1)], in_=psq[:, 0:512],
                                         func=AF.Identity, bias=bq_sb[:, 0:1], scale=1.0)
                    nc.scalar.activation(out=kT[:, 512 * n:512 * (n + 1)], in_=psq[:, 512:1024],
                                         func=AF.Identity, bias=bq_sb[:, 1:2], scale=1.0)
                    psqv = ps.tile([128, 512], F32, tag="tp", bufs=2, name=f"pqv{b}_{n}")
                    for k in range(8):
                        nc.tensor.matmul(psqv[:], wsb[k][2][:], xtk(k),
                                         start=(k == 0), stop=(k == 7))
                    vt = sb.tile([128, 512], F16, tag="vt", bufs=3, name=f"vt{b}_{n}")
                    nc.scalar.activation(out=vt[:], in_=psqv[:], func=AF.Identity,
                                         bias=bq_sb[:, 2:3], scale=1.0)

                    # k/v row-major blocks (present outputs; v also feeds av)
                    for r in range(4):
                        sbi = 4 * n + r
                        psk = ps.tile([128, 128], F16, tag="tp", bufs=2, name=f"psk{b}_{sbi}")
                        nc.tensor.transpose(psk[:], kT[:, 128 * sbi:128 * (sbi + 1)], ident16[:])
                        krow = sb.tile([128, 128], F32, tag="krow", bufs=4, name=f"kr{b}_{sbi}")
                        nc.vector.tensor_copy(krow[:], psk[:])
                        nc.sync.dma_start(
                            pres_ext[0, b, :, 128 * sbi:128 * (sbi + 1), :].rearrange("h s d -> s h d"),
                            krow[:].rearrange("p (h d) -> p h d", h=2))
                        psv2 = ps.tile([128, 128], F16, tag="tp", bufs=2, name=f"psv{b}_{sbi}")
                        nc.tensor.transpose(psv2[:], vt[:, 128 * r:128 * (r + 1)], ident16[:])
                        vrow = sb.tile([128, 128], F32, tag="krow", bufs=4, name=f"vr{b}_{sbi}")
                        nc.vector.tensor_copy(vrow[:], psv2[:])
                        nc.sync.dma_start(
                            pres_ext[1, b, :, 128 * sbi:128 * (sbi + 1), :].rearrange("h s d -> s h d"),
                            vrow[:].rearrange("p (h d) -> p h d", h=2))
                        vrow_b = sb.tile([128, 128], F16, tag="vrowb", bufs=32, name=f"vb{b}_{sbi}")
                        nc.vector.tensor_copy(vrow_b[:], psv2[:])
                        vrows.append(vrow_b)

                # ---- attention ----
                avT = sb.tile([128, S], F16, tag="avT", bufs=2, name=f"avT{b}")
                for g in range(4):
                    npair = (g + 2) // 2           # number of 1024-wide chunk pairs
                    for hh in range(2):
                        hs = 64 * hh
                        Es = {}
                        for r in range(4):
                            i = 4 * g + r
                            accs = []
                            for p in range(npair):
                                wid = min(1024, 512 * (g + 1) - 1024 * p)
                                pss = ps.tile([128, 1024], F32, tag="mm", bufs=2,
                                              name=f"pss{b}_{hh}_{i}_{p}")
                                for half in range(wid // 512):
                                    ch = 2 * p + half
                                    nc.tensor.matmul(
                                        pss[:, 512 * half:512 * (half + 1)],
                                        qT[hs:hs + 64, 128 * i:128 * (i + 1)],
                                        kT[hs:hs + 64, 512 * ch:512 * (ch + 1)],
                                        start=True, stop=True)
                                    if ch == g:
                                        nc.vector.tensor_add(
                                            pss[:, 512 * half:512 * (half + 1)],
                                            pss[:, 512 * half:512 * (half + 1)], masks[r][:])
                                E = sb.tile([128, 1024], F32R, tag="E", bufs=10,
                                            name=f"E{b}_{hh}_{i}_{p}")
                                acc = sb.tile([128, 1], F32, tag="acc", bufs=10,
                                              name=f"ac{b}_{hh}_{i}_{p}")
                                nc.scalar.activation(out=E[:, :wid], in_=pss[:, :wid],
                                                     func=AF.Exp, scale=0.125,
                                                     accum_out=acc[:])
                                Es[(r, p)] = E
                                accs.append(acc)
                            if len(accs) == 1:
                                rsum = accs[0]
                            else:
                                rsum = sb.tile([128, 1], F32, tag="rs", bufs=4,
                                               name=f"rs{b}_{hh}_{i}")
                                nc.vector.tensor_add(rsum[:], accs[0][:], accs[1][:])
                            rcp = sb.tile([128, 1], F32, tag="rcp", bufs=6,
                                          name=f"rc{b}_{hh}_{i}")
                            nc.vector.reciprocal(rcp[:], rsum[:])
                            for p in range(npair):
                                wid = min(1024, 512 * (g + 1) - 1024 * p)
                                E = Es[(r, p)]
                                nc.vector.tensor_scalar_mul(E[:, :wid], E[:, :wid], rcp[:])
                                nc.sync.dma_start(
                                    attn_ext[b, hh, 128 * i:128 * (i + 1),
                                             1024 * p:1024 * p + wid],
                                    E[:, :wid].bitcast(F32))
                        # attn @ v (transposed layout, fp16 matmul)
                        avp = ps.tile([64, 512], F32, tag="av", bufs=2, name=f"avp{b}_{hh}_{g}")
                        njs = 4 * (g + 1)
                        for j in range(njs):
                            p, sub = j // 8, j % 8
                            pst = ps.tile([128, 512], F32R, tag="tp", bufs=2,
                                          name=f"pt{b}_{hh}_{g}_{j}")
                            for r in range(4):
                                nc.tensor.transpose(
                                    pst[:, 128 * r:128 * (r + 1)],
                                    Es[(r, p)][:, 128 * sub:128 * (sub + 1)], ident_r[:])
                            R = sb.tile([128, 512], F16, tag="R", bufs=3,
                                        name=f"R{b}_{hh}_{g}_{j}")
                            if j % 2 == 0:
                                nc.scalar.copy(R[:], pst[:])
                            else:
                                nc.vector.tensor_copy(R[:], pst[:])
                            nc.tensor.matmul(avp[:], vrows[j][:, hs:hs + 64], R[:],
                                             start=(j == 0), stop=(j == njs - 1))
                        nc.scalar.copy(avT[hs:hs + 64, 512 * g:512 * (g + 1)], avp[:])
                    # ---- partial projection for this row group (fp16) ----
                    for mi in range(4):
                        m = 4 * g + mi
                        psp = ps.tile([128, 1024], F32, tag="mm", bufs=2,
                                      name=f"pp{b}_{g}_{mi}")
                        for nn2 in range(2):
                            nc.tensor.matmul(psp[:, 512 * nn2:512 * (nn2 + 1)],
                                             avT[:, 128 * m:128 * (m + 1)],
                                             wp16[:, 512 * nn2:512 * (nn2 + 1)],
                                             start=True, stop=False)
                            nc.tensor.matmul(psp[:, 512 * nn2:512 * (nn2 + 1)],
                                             ones16[:],
                                             bp16[:, 512 * nn2:512 * (nn2 + 1)],
                                             start=False, stop=True)
                        ao = sb.tile([128, 1024], F32, tag="ao", bufs=4,
                                     name=f"ao{b}_{g}_{mi}")
                        if mi % 2 == 0:
                            nc.scalar.copy(ao[:], psp[:])
                        else:
                            nc.vector.tensor_copy(ao[:], psp[:])
                        nc.sync.dma_start(apart_ext[b, 128 * m:128 * (m + 1), :], ao[:])
    nc.compile()
    return nc


def _get_nc():
    if "nc" not in _CACHE:
        _CACHE["nc"] = _build()
    return _CACHE["nc"]


def kernel(x, w_attn, b_attn, w_proj, b_proj):
    global _last_in_maps
    x = np.asarray(x, dtype=np.float32)
    w_attn = np.asarray(w_attn, dtype=np.float32)
    b_attn = np.asarray(b_attn, dtype=np.float32)
    w_proj = np.asarray(w_proj, dtype=np.float32)
    b_proj = np.asarray(b_proj, dtype=np.float32)

    nc = _get_nc()
    x16 = np.ascontiguousarray(x.astype(np.float16))
    in_maps = []
    for c in range(N_CORES):
        lo, hi = PD * c, PD * (c + 1)
        wqkv = np.ascontiguousarray(np.concatenate(
            [w_attn[:, lo:hi], w_attn[:, D + lo:D + hi], w_attn[:, 2 * D + lo:2 * D + hi]],
            axis=1).astype(np.float16))
        bqkv = np.ascontiguousarray(np.stack(
            [b_attn[lo:hi], b_attn[D + lo:D + hi], b_attn[2 * D + lo:2 * D + hi]]))
        in_maps.append({
            "x16": x16,
            "wqkv16": wqkv,
            "bqkv": bqkv,
            "wp16": np.ascontiguousarray(w_proj[lo:hi, :].astype(np.float16)),
            "bp16": (b_proj / N_CORES)[None, :].astype(np.float16).copy(),
        })

    _last_in_maps = in_maps
    res = run_bass_kernel_spmd(nc, in_maps, list(range(N_CORES)))
    rs = res.results

    attn = np.concatenate([r["attn"] for r in rs], axis=1)          # [B, H, S, S]
    present = np.concatenate([r["present"] for r in rs], axis=2)    # [2, B, H, S, HD]
    a = rs[0]["a_part"]
    for r in rs[1:]:
        a = a + r["a_part"]
    return a, present, attn


# revision 9
# speedup vs baseline: 1.2161x; 1.1619x over previous
"""Trainium2 Bass kernel for GPT-style attention block (B=2, S=2048, D=1024, H=16).

Sharding: tensor-parallel over heads, 2 heads per core (8 cores).
Each core computes qkv for its heads, causal softmax attention, its partial
output projection (contracting only its 128 head-dims); host sums the 8
partial projections and concatenates head-sharded attn/present outputs.

Precision: fp16 matmul operands (1 cyc/row on the PE, ~5e-4 rel err, fp32
PSUM accumulation); the softmax/attn-output path runs in fp32 (float32r).
Causal structure: only lower-triangular 512-col chunks are computed; the
strictly-upper region relies on pre-zeroed output buffers.
"""
import os
import numpy as np

import concourse.bacc as bacc
import concourse.mybir as mybir
import concourse.tile as tile
from concourse.bass_utils import run_bass_kernel_spmd
from concourse.masks import make_identity

B, S, D, H = 2, 2048, 1024, 16
HD = D // H            # 64
N_CORES = 8
HPC = H // N_CORES     # 2 heads per core
PD = HPC * HD          # 128 partition dims per core
F32 = mybir.dt.float32
F32R = mybir.dt.float32r
F16 = mybir.dt.float16
AF = mybir.ActivationFunctionType

_CACHE = {}
_last_in_maps = None


def _build():
    nc = bacc.Bacc(None, target_bir_lowering=False)

    x_ext = nc.declare_dram_parameter("x16", [B, S, D], F16, isOutput=False)
    wqkv_ext = nc.declare_dram_parameter("wqkv16", [D, 3 * PD], F16, isOutput=False)
    bqkv_ext = nc.declare_dram_parameter("bqkv", [3, PD], F32, isOutput=False)
    wp_ext = nc.declare_dram_parameter("wp16", [PD, D], F16, isOutput=False)
    attn_ext = nc.declare_dram_parameter("attn", [B, HPC, S, S], F32, isOutput=True)
    pres_ext = nc.declare_dram_parameter("present", [2, B, HPC, S, HD], F32, isOutput=True)
    apart_ext = nc.declare_dram_parameter("a_part", [B, S, D], F32, isOutput=True)

    with tile.TileContext(nc) as tc:
        with (
            tc.tile_pool(name="cst", bufs=1) as cst,
            tc.tile_pool(name="sb", bufs=1) as sb,
            tc.tile_pool(name="ps", bufs=1, space="PSUM") as ps,
        ):
            # ---- constants ----
            ident = cst.tile([128, 128], F32)
            make_identity(nc, ident[:])
            ident16 = cst.tile([128, 128], F16)
            nc.vector.tensor_copy(ident16[:], ident[:])

            # causal masks for the diagonal 512-chunk, one per row-block
            # position p in its group: keep where col' <= 128p + row'
            masks = []
            for p in range(4):
                m = cst.tile([128, 512], F32, name=f"mask{p}")
                nc.gpsimd.memset(m[:], 0.0)
                nc.gpsimd.affine_select(
                    out=m[:], in_=m[:],
                    compare_op=mybir.AluOpType.is_ge,
                    fill=-1e9, base=128 * p,
                    pattern=[[-1, 512]], channel_multiplier=1,
                )
                masks.append(m)

            # qkv weights: 8 k-blocks x 3 col-tiles (fp16, direct DMA)
            wsb = [[None] * 3 for _ in range(8)]
            for k in range(8):
                for t in range(3):
                    wr = cst.tile([128, 128], F16, name=f"w{k}_{t}")
                    nc.sync.dma_start(wr[:], wqkv_ext[128 * k:128 * (k + 1), 128 * t:128 * (t + 1)])
                    wsb[k][t] = wr

            bq_sb = cst.tile([128, 3], F32)
            for t in range(3):
                nc.sync.dma_start(bq_sb[:, t:t + 1], bqkv_ext[t][:, None])

            wp16 = cst.tile([PD, D], F16)
            nc.sync.dma_start(wp16[:], wp_ext[:])

            for b in range(B):
                # ---- qkv projection (transposed layout) ----
                qT = sb.tile([128, S], F16, tag="qT", bufs=2, name=f"qT{b}")
                kT = sb.tile([128, S], F16, tag="kT", bufs=2, name=f"kT{b}")
                vrows = []
                for n in range(4):
                    xrows = []
                    for r in range(4):
                        xr = sb.tile([128, D], F16, tag="xrow", bufs=6, name=f"xr{b}_{n}_{r}")
                        nc.sync.dma_start(xr[:], x_ext[b, 512 * n + 128 * r:512 * n + 128 * (r + 1), :])
                        xrows.append(xr)
                    xts = []
                    for kp in range(4):          # pairs of d-blocks
                        pst = ps.tile([128, 1024], F16, tag="mm", bufs=2, name=f"pxt{b}_{n}_{kp}")
                        for half in range(2):
                            k = 2 * kp + half
                            for r in range(4):
                                nc.tensor.transpose(
                                    pst[:, 512 * half + 128 * r:512 * half + 128 * (r + 1)],
                                    xrows[r][:, 128 * k:128 * (k + 1)], ident16[:])
                        xt = sb.tile([128, 1024], F16, tag="xt", bufs=6, name=f"xt{b}_{n}_{kp}")
                        nc.vector.tensor_copy(xt[:], pst[:])
                        xts.append(xt)

                    def xtk(k):
                        return xts[k // 2][:, 512 * (k % 2):512 * (k % 2 + 1)]

                    # q,k into qT/kT (paired psum); v into vt chunk
                    psq = ps.tile([128, 1024], F32, tag="mm", bufs=2, name=f"pqa{b}_{n}")
                    for t in range(2):
                        for k in range(8):
                            nc.tensor.matmul(psq[:, 512 * t:512 * (t + 1)],
                                             wsb[k][t][:], xtk(k),
                                             start=(k == 0), stop=(k == 7))
                    nc.vector.tensor_scalar_add(qT[:, 512 * n:512 * (n + 1)], psq[:, 0:512],
                                                bq_sb[:, 0:1])
                    nc.vector.tensor_scalar_add(kT[:, 512 * n:512 * (n + 1)], psq[:, 512:1024],
                                                bq_sb[:, 1:2])
                    psqv = ps.tile([128, 512], F32, tag="tp", bufs=2, name=f"pqv{b}_{n}")
                    for k in range(8):
                        nc.tensor.matmul(psqv[:], wsb[k][2][:], xtk(k),
                                         start=(k == 0), stop=(k == 7))
                    vt = sb.tile([128, 512], F16, tag="vt", bufs=3, name=f"vt{b}_{n}")
                    nc.vector.tensor_scalar_add(vt[:], psqv[:], bq_sb[:, 2:3])

                    # k/v row-major blocks (present outputs; v also feeds av)
                    for r in range(4):
                        sbi = 4 * n + r
                        psk = ps.tile([128, 128], F16, tag="tp", bufs=2, name=f"psk{b}_{sbi}")
                        nc.tensor.transpose(psk[:], kT[:, 128 * sbi:128 * (sbi + 1)], ident16[:])
                        krow = sb.tile([128, 128], F32, tag="krow", bufs=4, name=f"kr{b}_{sbi}")
                        nc.vector.tensor_copy(krow[:], psk[:])
                        nc.sync.dma_start(
                            pres_ext[0, b, :, 128 * sbi:128 * (sbi + 1), :].rearrange("h s d -> s h d"),
                            krow[:].rearrange("p (h d) -> p h d", h=2))
                        psv2 = ps.tile([128, 128], F16, tag="tp", bufs=2, name=f"psv{b}_{sbi}")
                        nc.tensor.transpose(psv2[:], vt[:, 128 * r:128 * (r + 1)], ident16[:])
                        vrow = sb.tile([128, 128], F32, tag="krow", bufs=4, name=f"vr{b}_{sbi}")
                        nc.vector.tensor_copy(vrow[:], psv2[:])
                        nc.sync.dma_start(
                            pres_ext[1, b, :, 128 * sbi:128 * (sbi + 1), :].rearrange("h s d -> s h d"),
                            vrow[:].rearrange("p (h d) -> p h d", h=2))
                        vrow_b = sb.tile([128, 128], F16, tag="vrowb", bufs=32, name=f"vb{b}_{sbi}")
                        nc.vector.tensor_copy(vrow_b[:], psv2[:])
                        vrows.append(vrow_b)

                # ---- attention ----
                avT = sb.tile([128, S], F16, tag="avT", bufs=2, name=f"avT{b}")
                for g in range(4):
                    npair = (g + 2) // 2           # number of 1024-wide chunk pairs
                    for hh in range(2):
                        hs = 64 * hh
                        Es = {}
                        for r in range(4):
                            i = 4 * g + r
                            accs = []
                            for p in range(npair):
                                wid = min(1024, 512 * (g + 1) - 1024 * p)
                                pss = ps.tile([128, 1024], F32, tag="mm", bufs=2,
                                              name=f"pss{b}_{hh}_{i}_{p}")
                                for half in range(wid // 512):
                                    ch = 2 * p + half
                                    nc.tensor.matmul(
                                        pss[:, 512 * half:512 * (half + 1)],
                                        qT[hs:hs + 64, 128 * i:128 * (i + 1)],
                                        kT[hs:hs + 64, 512 * ch:512 * (ch + 1)],
                                        start=True, stop=True)
                                    if ch == g:
                                        nc.vector.tensor_add(
                                            pss[:, 512 * half:512 * (half + 1)],
                                            pss[:, 512 * half:512 * (half + 1)], masks[r][:])
                                E = sb.tile([128, 1024], F16, tag="E", bufs=10,
                                            name=f"E{b}_{hh}_{i}_{p}")
                                acc = sb.tile([128, 1], F32, tag="acc", bufs=10,
                                              name=f"ac{b}_{hh}_{i}_{p}")
                                nc.scalar.activation(out=E[:, :wid], in_=pss[:, :wid],
                                                     func=AF.Exp, scale=0.125,
                                                     accum_out=acc[:])
                                Es[(r, p)] = E
                                accs.append(acc)
                            if len(accs) == 1:
                                rsum = accs[0]
                            else:
                                rsum = sb.tile([128, 1], F32, tag="rs", bufs=4,
                                               name=f"rs{b}_{hh}_{i}")
                                nc.vector.tensor_add(rsum[:], accs[0][:], accs[1][:])
                            rcp = sb.tile([128, 1], F32, tag="rcp", bufs=6,
                                          name=f"rc{b}_{hh}_{i}")
                            nc.vector.reciprocal(rcp[:], rsum[:])
                            for p in range(npair):
                                wid = min(1024, 512 * (g + 1) - 1024 * p)
                                E = Es[(r, p)]
                                nc.vector.tensor_scalar_mul(E[:, :wid], E[:, :wid], rcp[:])
                                Ef = sb.tile([128, 1024], F32, tag="Ef", bufs=4,
                                             name=f"Ef{b}_{hh}_{i}_{p}")
                                nc.vector.tensor_copy(Ef[:, :wid], E[:, :wid])
                                nc.sync.dma_start(
                                    attn_ext[b, hh, 128 * i:128 * (i + 1),
                                             1024 * p:1024 * p + wid],
                                    Ef[:, :wid])
                        # attn @ v (transposed layout, fp16 matmul)
                        avp = ps.tile([64, 512], F32, tag="av", bufs=2, name=f"avp{b}_{hh}_{g}")
                        njs = 4 * (g + 1)
                        for j in range(njs):
                            p, sub = j // 8, j % 8
                            pst = ps.tile([128, 512], F16, tag="tp", bufs=2,
                                          name=f"pt{b}_{hh}_{g}_{j}")
                            for r in range(4):
                                nc.tensor.transpose(
                                    pst[:, 128 * r:128 * (r + 1)],
                                    Es[(r, p)][:, 128 * sub:128 * (sub + 1)], ident16[:])
                            R = sb.tile([128, 512], F16, tag="R", bufs=3,
                                        name=f"R{b}_{hh}_{g}_{j}")
                            if j % 2 == 0:
                                nc.scalar.copy(R[:], pst[:])
                            else:
                                nc.vector.tensor_copy(R[:], pst[:])
                            nc.tensor.matmul(avp[:], vrows[j][:, hs:hs + 64], R[:],
                                             start=(j == 0), stop=(j == njs - 1))
                        nc.scalar.copy(avT[hs:hs + 64, 512 * g:512 * (g + 1)], avp[:])
                    # ---- partial projection for this row group (fp16) ----
                    for mi in range(4):
                        m = 4 * g + mi
                        psp = ps.tile([128, 1024], F32, tag="mm", bufs=2,
                                      name=f"pp{b}_{g}_{mi}")
                        for nn2 in range(2):
                            nc.tensor.matmul(psp[:, 512 * nn2:512 * (nn2 + 1)],
                                             avT[:, 128 * m:128 * (m + 1)],
                                             wp16[:, 512 * nn2:512 * (nn2 + 1)],
                                             start=True, stop=True)
                        ao = sb.tile([128, 1024], F32, tag="ao", bufs=4,
                                     name=f"ao{b}_{g}_{mi}")
                        if mi % 2 == 0:
                            nc.scalar.copy(ao[:], psp[:])
                        else:
                            nc.vector.tensor_copy(ao[:], psp[:])
                        nc.sync.dma_start(apart_ext[b, 128 * m:128 * (m + 1), :], ao[:])
    nc.compile()
    return nc


def _get_nc():
    if "nc" not in _CACHE:
        _CACHE["nc"] = _build()
    return _CACHE["nc"]


def kernel(x, w_attn, b_attn, w_proj, b_proj):
    global _last_in_maps
    x = np.asarray(x, dtype=np.float32)
    w_attn = np.asarray(w_attn, dtype=np.float32)
    b_attn = np.asarray(b_attn, dtype=np.float32)
    w_proj = np.asarray(w_proj, dtype=np.float32)
    b_proj = np.asarray(b_proj, dtype=np.float32)

    nc = _get_nc()
    x16 = np.ascontiguousarray(x.astype(np.float16))
    in_maps = []
    for c in range(N_CORES):
        lo, hi = PD * c, PD * (c + 1)
        wqkv = np.ascontiguousarray(np.concatenate(
            [w_attn[:, lo:hi], w_attn[:, D + lo:D + hi], w_attn[:, 2 * D + lo:2 * D + hi]],
            axis=1).astype(np.float16))
        bqkv = np.ascontiguousarray(np.stack(
            [b_attn[lo:hi], b_attn[D + lo:D + hi], b_attn[2 * D + lo:2 * D + hi]]))
        in_maps.append({
            "x16": x16,
            "wqkv16": wqkv,
            "bqkv": bqkv,
            "wp16": np.ascontiguousarray(w_proj[lo:hi, :].astype(np.float16)),
        })

    _last_in_maps = in_maps
    res = run_bass_kernel_spmd(nc, in_maps, list(range(N_CORES)))
    rs = res.results

    attn = np.concatenate([r["attn"] for r in rs], axis=1)          # [B, H, S, S]
    present = np.concatenate([r["present"] for r in rs], axis=2)    # [2, B, H, S, HD]
    a = rs[0]["a_part"]
    for r in rs[1:]:
        a = a + r["a_part"]
    a = a + b_proj
    return a, present, attn
